# revision 1
# baseline (speedup 1.0000x reference)
"""GroupWiseLinearProjector Trainium2 kernel.

out[b, o, h, w] = sum_c x[b, c, h, w] * Wg[(h%4)*4 + (w%4), o, c]

Strategy: data-parallel over batch (16 batches -> 2 per NeuronCore, 8 cores).
Weights are host-rearranged so each m-tile's weights load as one contiguous
DMA. Phase-interleaved pixels are selected directly by strided access
patterns in the matmul rhs and in the PSUM->SBUF scatter copy, so no
gather/rearrange pass is needed on-chip.

Per-core loop (full unroll, Tile handles scheduling/semaphores):
  load x k-tiles [128c, 2b, 64h, 64w] (resident whole kernel)
  for mo in 4 m-tiles:
    load weights [128c, 16g, 4kc, 128o] (one contiguous DMA)
    for g in 16 phases:
      psum[128o, 2b, 16h4, 16w4] = sum_kc W[g,kc,mo].T @ x[kc][:, b, r::4, q::4]
      scatter-copy psum -> out staging [128o, 2b, 64h, 64w] (strided dest)
    DMA out staging -> DRAM
"""

import numpy as np

B, CS, CT, H, W = 16, 512, 512, 64, 64
NCORES = 8
BPC = B // NCORES  # batches per core
KT = CS // 128  # 4 k-tiles
MT = CT // 128  # 4 m-tiles

# dtype configuration: "fp16" (fast, ~5e-4 rel err) or "fp32r" (exact fp32 bits)
DTYPE_MODE = "fp16"
OUT_FP16 = True  # store output as fp16 (halves output traffic)


def _build_nc(mode, out_fp16):
    import concourse.bass as bass
    import concourse.tile as tile
    from concourse import mybir

    in_dt = mybir.dt.float16 if mode == "fp16" else mybir.dt.float32
    mm_dt = mybir.dt.float16 if mode == "fp16" else mybir.dt.float32r
    out_dt = mybir.dt.float16 if out_fp16 else mybir.dt.float32

    nc = bass.Bass()
    x_ext = nc.declare_dram_parameter("x", [BPC, CS, H, W], in_dt, isOutput=False)
    w_ext = nc.declare_dram_parameter("w", [MT, 128, 16, KT, 128], in_dt, isOutput=False)
    out_ext = nc.declare_dram_parameter("out", [BPC, CT, H, W], out_dt, isOutput=True)

    # fp16 fits all BPC batches of x in SBUF at once (one batch group);
    # fp32 needs per-batch groups (weights re-streamed per group).
    GB = BPC if mode == "fp16" else 1  # batches per group
    NG = BPC // GB  # number of groups

    with tile.TileContext(nc) as tc:
        with (
            tc.tile_pool(name="xpool", bufs=1) as xpool,
            tc.tile_pool(name="wpool", bufs=2) as wpool,
            tc.tile_pool(name="opool", bufs=2) as opool,
            tc.tile_pool(name="psum", bufs=4, space=bass.MemorySpace.PSUM) as pp,
        ):
            for grp in range(NG):
                b0 = grp * GB
                xk = []
                for kc in range(KT):
                    xt = xpool.tile([128, GB, H, W], in_dt, tag=f"x{kc}")
                    nc.sync.dma_start(
                        out=xt[:],
                        in_=x_ext[
                            b0 : b0 + GB, kc * 128 : (kc + 1) * 128
                        ].rearrange("b c h w -> c b h w"),
                    )
                    xk.append(xt)

                for mo in range(MT):
                    wm = wpool.tile([128, 16, KT, 128], in_dt, tag="w")
                    nc.sync.dma_start(out=wm[:], in_=w_ext[mo])
                    om = opool.tile([128, GB, H, W], out_dt, tag="o")
                    for g in range(16):
                        r, q = g // 4, g % 4
                        ps = pp.tile([128, GB, 16, 16], mybir.dt.float32)
                        for b in range(GB):
                            for kc in range(KT):
                                lhsT = wm[:, g, kc, :]
                                rhs = xk[kc][:, b, r::4, q::4]
                                if mode == "fp32r":
                                    lhsT = lhsT.bitcast(mm_dt)
                                    rhs = rhs.bitcast(mm_dt)
                                nc.tensor.matmul(
                                    ps[:, b],
                                    lhsT,
                                    rhs,
                                    start=(kc == 0),
                                    stop=(kc == KT - 1),
                                )
                        nc.vector.tensor_copy(om[:, :, r::4, q::4], ps[:])
                    nc.sync.dma_start(
                        out=out_ext[
                            b0 : b0 + GB, mo * 128 : (mo + 1) * 128
                        ].rearrange("b o h w -> o b h w"),
                        in_=om[:],
                    )
                    # observer: tiny DVE write into the staging tile AFTER the
                    # out-DMA read. The WAR dep makes the DVE stream observe
                    # the DMA's completion lane, collapsing the tail drain's
                    # (and slot-reuse copies') wait sets to a single DVE wait.
                    nc.vector.memset(om[0:1, 0, 0:1, 0:1], 0.0)
    return nc


def _strip_redundant_waits(nc):
    """Walrus's MM and pseudo-DMA instruction structs support a single
    sync-wait command, but Tile emits 2-3 on slot-reuse boundaries. Most are
    transitively implied by another wait on the same instruction (Tile's sem
    assignment is per-proc minimal but not transitively minimal). Compute a
    happens-before closure and reduce every multi-wait instruction to one
    wait, verifying coverage.

    Soundness: knowledge of a wait (S >= v) = completion knowledge of the
    instruction whose cumulative increment brings S to >= v. An instruction's
    completion implies: its own waits held, its own incs fired, and - for
    in-order compute engines (completion is pc-monotone; DMA completions are
    async so DMAs are excluded) - completion of all pc-earlier same-engine
    instructions.
    """
    f = nc.m.functions[0]
    insts = []
    for blk in f.blocks:
        for inst in blk.instructions:
            insts.append(inst)

    sem_incs = {}  # sem -> list of (cum_value, inst_idx)
    for idx, inst in enumerate(insts):
        si = inst.sync_info
        if si is None:
            continue
        for u in si.on_update:
            if u.update_mode not in ("sem-inc", "sem-add-imm"):
                continue
            lst = sem_incs.setdefault(u.ant_name, [])
            prev = lst[-1][0] if lst else 0
            lst.append((prev + u.update_value, idx))

    def incer_of(sem, val):
        for cum, idx in sem_incs.get(sem, []):
            if cum >= val:
                return idx
        return None

    know = [dict() for _ in insts]  # completion knowledge: sem -> value

    def join(dst, src):
        changed = False
        for s, v in src.items():
            if dst.get(s, 0) < v:
                dst[s] = v
                changed = True
        return changed

    is_dma = [type(i).__name__ == "InstDMACopy" for i in insts]
    for _ in range(4):
        changed = False
        stream_know = {}  # engine -> accumulated completion knowledge
        for idx, inst in enumerate(insts):
            si = inst.sync_info
            k = know[idx]
            if si is not None:
                for w in si.on_wait:
                    if w.wait_mode != "sem-ge-imm":
                        continue
                    changed |= join(k, {w.ant_name: w.wait_value})
                    src = incer_of(w.ant_name, w.wait_value)
                    if src is not None:
                        changed |= join(k, know[src])

            eng = str(getattr(inst, "engine", None))
            if not is_dma[idx]:
                sk = stream_know.setdefault(eng, {})
                changed |= join(k, sk)
                join(sk, k)
        if not changed:
            break

    def wait_knowledge(w):
        k = {w.ant_name: w.wait_value}
        src = incer_of(w.ant_name, w.wait_value)
        if src is not None:
            for s, v in know[src].items():
                if k.get(s, 0) < v:
                    k[s] = v
        return k

    from itertools import combinations

    # sem -> engine of its (sole) updater stream; None if mixed or DMA-updated
    sem_engine = {}
    for idx, inst in enumerate(insts):
        si = inst.sync_info
        if si is None:
            continue
        eng = None if is_dma[idx] else str(getattr(inst, "engine", None))
        for u in si.on_update:
            if u.ant_name in sem_engine and sem_engine[u.ant_name] != eng:
                sem_engine[u.ant_name] = None
            else:
                sem_engine.setdefault(u.ant_name, eng)

    inst_pos = {id(inst): idx for idx, inst in enumerate(insts)}

    def droppable_by_stream_order(inst, w):
        # A wait on the instruction's own engine's completion sem whose incer
        # precedes it in the same strict-FIFO stream is satisfied by in-order
        # execution.
        eng = str(getattr(inst, "engine", None))
        if sem_engine.get(w.ant_name) != eng or eng == "None":
            return False
        ix = inst_pos[id(inst)]
        best = 0
        for cum, idx in sem_incs.get(w.ant_name, []):
            if idx < ix:
                best = cum
            else:
                break
        return best >= w.wait_value

    def reduce_waits(inst, max_keep):
        si = inst.sync_info
        waits = [
            w for w in si.on_wait if not droppable_by_stream_order(inst, w)
        ]
        if len(waits) < len(si.on_wait):
            inst.sync_info = type(si)(
                on_wait=waits, on_update=list(si.on_update)
            )
            si = inst.sync_info
        if len(waits) <= max_keep:
            return True
        for n_keep in range(1, max_keep + 1):
            for kept in combinations(waits, n_keep):
                kk = {}
                for w in kept:
                    join(kk, wait_knowledge(w))
                if all(
                    kk.get(d.ant_name, 0) >= d.wait_value
                    for d in waits
                    if d not in kept
                ):
                    inst.sync_info = type(si)(
                        on_wait=list(kept), on_update=list(si.on_update)
                    )
                    return True
        return False

    for inst in insts:
        si = inst.sync_info
        if si is None or len(si.on_wait) <= 1:
            continue
        tn = type(inst).__name__
        limit = 6 if tn == "InstDrain" else 1
        if not reduce_waits(inst, limit):
            if tn in ("InstMatmult", "InstDMACopy"):
                raise RuntimeError(
                    f"{tn} {inst.name} has irreducible waits: "
                    f"{[(w.ant_name, w.wait_value) for w in inst.sync_info.on_wait]}"
                )


_NC_CACHE = {}


def _get_nc(mode, out_fp16):
    key = (mode, out_fp16)
    if key not in _NC_CACHE:
        nc = _build_nc(mode, out_fp16)
        _strip_redundant_waits(nc)
        _NC_CACHE[key] = nc
    return _NC_CACHE[key]


def _prep_inputs(x, Wg, mode):
    np_dt = np.float16 if mode == "fp16" else np.float32
    # W_dma[mo, p, g, kc, o] = Wg[g, mo*128+o, kc*128+p]
    W5 = Wg.reshape(16, MT, 128, KT, 128)  # [g, mo, o, kc, p]
    W_dma = np.ascontiguousarray(W5.transpose(1, 4, 0, 3, 2), dtype=np_dt)
    xs = np.ascontiguousarray(x, dtype=np_dt)
    in_maps = [
        {"x": xs[i * BPC : (i + 1) * BPC], "w": W_dma} for i in range(NCORES)
    ]
    return in_maps


def run(x, Wg, mode=None, out_fp16=None, trace=False):
    from concourse.bass_utils import run_bass_kernel_spmd

    if mode is None:
        mode = DTYPE_MODE
    if out_fp16 is None:
        out_fp16 = OUT_FP16
    nc = _get_nc(mode, out_fp16)
    in_maps = _prep_inputs(x, Wg, mode)
    res = run_bass_kernel_spmd(nc, in_maps, list(range(NCORES)), trace=trace)
    out = np.concatenate([res.results[i]["out"] for i in range(NCORES)], axis=0)
    return out.astype(np.float32, copy=False), res


def kernel(x, Wg):
    out, _ = run(x, Wg)
    return out

